# revision 4
# baseline (speedup 1.0000x reference)
"""Trainium2 Bass kernel for nn_Diffusion: y = expm(-t*L) @ x.

Math: ||t*L||_2 ~= 0.2 for the target inputs (L is PSD with eigenvalues
roughly in [0, 0.4], t = 0.5), so the action of the matrix exponential is
computed with a degree-D Taylor series applied as chained matvecs:

    y = sum_{k=0..D} (-t)^k/k! L^k x,   v_0 = x,  v_k = (-t/k) * L @ v_{k-1}

Sharding: x is split column-wise (channel-parallel) across the 8 cores (64
channels each); L is replicated. No cross-core communication.

Per-core compute (transposed orientation, so the PE streams N=512-wide):
    v'^T = v^T @ L  computed as  out = lhsT.T @ rhs  with lhsT = v row-major
    tiles [128, 64] and rhs = L row-blocks [128, 512].
Full fp32 accuracy at bf16 PE speed via a hi/lo split of both operands:
    L = L_hi + L_lo (bf16 pair, host-prepared), v = v_hi + v_lo (bf16 pair),
    L@v ~= L_hi v_hi + L_hi v_lo + L_lo v_hi  (3 bf16 products, fp32 PSUM).
The two free PE column halves are used concurrently (tile_position col
packing): col group g computes output chunk n = 2g + j during "j-pass" j.

Between terms, each j-pass's channel-major result [128, 512] is scaled out
of PSUM (ACT), split to bf16 hi/lo (DVE), and DMA-xbar-transposed back to
row-major [128, 4, 64] tile slices for the next term's stationary operand.
The next term's contraction visits the k-tiles produced by its
predecessor's j0-pass first, so the j1-pass post-processing overlaps the
next term's matmuls. y accumulates row-major in fp32 and is DMA'd out
directly.
"""

import os
import sys

for _p in ("/opt/trn_rl_repo", "/root/.axon_site/_ro/trn_rl_repo"):
    if os.path.isdir(_p) and _p not in sys.path:
        sys.path.insert(0, _p)

from contextlib import ExitStack

import ml_dtypes
import numpy as np

import concourse.bacc as bacc
import concourse.mybir as mybir
import concourse.tile as tile
from concourse.bass_utils import run_bass_kernel_spmd

BF16 = ml_dtypes.bfloat16
N = 2048
C = 512
N_CORES = 8
CS = C // N_CORES  # 64 channels per core
KT = N // 128  # 16 contraction tiles
D = 4  # Taylor degree (rel err ~6.5e-7 for t*||L|| ~= 0.2)
CHUNK = 512

_cache: dict = {}
last_result = None  # BassKernelResults of the most recent run (for test.py)

# k-tile visit order for terms > 1: tiles produced by the previous term's
# j0-pass (rows 0-511 and 1024-1535 -> tiles 0-3 and 8-11) first, so the
# j1-pass transposes can complete while these are being consumed.
KK_BATCH_A = [0, 1, 2, 3, 8, 9, 10, 11]
KK_BATCH_B = [4, 5, 6, 7, 12, 13, 14, 15]


def _build(t: float):
    f32 = mybir.dt.float32
    bf16 = mybir.dt.bfloat16
    nc = bacc.Bacc(
        "TRN2", target_bir_lowering=False, debug=False, num_devices=N_CORES
    )
    x_d = nc.dram_tensor("x", [N, CS], f32, kind="ExternalInput").ap()
    Lhi_d = nc.dram_tensor("L_hi", [N, N], bf16, kind="ExternalInput").ap()
    Llo_d = nc.dram_tensor("L_lo", [N, N], bf16, kind="ExternalInput").ap()
    y_d = nc.dram_tensor("y", [N, CS], f32, kind="ExternalOutput").ap()

    with ExitStack() as ctx:
        tc = ctx.enter_context(tile.TileContext(nc))
        Lp = ctx.enter_context(tc.tile_pool(name="L", bufs=1))
        vp = ctx.enter_context(tc.tile_pool(name="v", bufs=2))
        sp = ctx.enter_context(tc.tile_pool(name="s", bufs=3))
        yp = ctx.enter_context(tc.tile_pool(name="yp", bufs=1))
        pp = ctx.enter_context(tc.tile_pool(name="ps", bufs=2, space="PSUM"))

        Lhi = Lp.tile([128, KT, N], bf16, tag="Lhi")
        Llo = Lp.tile([128, KT, N], bf16, tag="Llo")
        x_sb = yp.tile([128, KT, CS], f32, tag="xsb")
        y_rm = yp.tile([128, KT, CS], f32, tag="y")

        nc.sync.dma_start(x_sb[:], x_d.rearrange("(k p) c -> p k c", p=128))
        # L in 4 MB transfers (8 k-tiles each) to amortize per-DMA latency;
        # L_hi first (2 of the 3 products stream it), then L_lo.
        for Ld, Lsb in ((Lhi_d, Lhi), (Llo_d, Llo)):
            for h in (0, 1):
                nc.sync.dma_start(
                    Lsb[:, 8 * h : 8 * (h + 1), :],
                    Ld[1024 * h : 1024 * (h + 1), :].rearrange(
                        "(k p) c -> p k c", p=128
                    ),
                )

        # v_0 = x as a bf16 hi/lo pair; y starts as the exact fp32 x.
        vhi = vp.tile([128, KT, CS], bf16, tag="vhi")
        vlo = vp.tile([128, KT, CS], bf16, tag="vlo")
        nc.vector.tensor_copy(vhi[:], x_sb[:])
        nc.vector.tensor_sub(vlo[:], x_sb[:], vhi[:])
        nc.scalar.copy(y_rm[:], x_sb[:])

        for k in range(1, D + 1):
            s_k = float(-t / k)
            # (pi, kk) emission order: batch A tiles first for terms > 1.
            batches = [list(range(KT))] if k == 1 else [KK_BATCH_A, KK_BATCH_B]
            seq = [
                (pi, kk)
                for batch in batches
                for pi in range(3)
                for kk in batch
            ]
            vhi_new = vp.tile([128, KT, CS], bf16, tag="vhi")
            vlo_new = vp.tile([128, KT, CS], bf16, tag="vlo")
            prods = [(vhi, Lhi), (vlo, Lhi), (vhi, Llo)]
            for j in (0, 1):
                ps = pp.tile([128, CHUNK], f32, tag=f"ps{j}")
                for idx, (pi, kk) in enumerate(seq):
                    vt, Lt = prods[pi]
                    for g in (0, 1):
                        n = 2 * g + j
                        nc.tensor.matmul(
                            ps[64 * g : 64 * (g + 1), :],
                            vt[:, kk, :],
                            Lt[:, kk, CHUNK * n : CHUNK * (n + 1)],
                            start=(idx == 0),
                            stop=(idx == len(seq) - 1),
                            tile_position=(0, 64 * g),
                            # The two col-groups' accumulation groups share
                            # a PSUM bank on disjoint partition ranges; the
                            # sim's zero-region tracker is partition-blind.
                            skip_group_check=True,
                        )
                # Post-process pass j: scale out of PSUM, split to bf16
                # hi/lo (still channel-major), transpose to row-major.
                yT = sp.tile([128, CHUNK], f32, tag="yT")
                nc.scalar.mul(yT[:], ps[:], s_k)
                hiT = sp.tile([128, CHUNK], bf16, tag="hiT")
                loT = sp.tile([128, CHUNK], bf16, tag="loT")
                nc.vector.tensor_copy(hiT[:], yT[:])
                nc.vector.tensor_sub(loT[:], yT[:], hiT[:])
                for g in (0, 1):
                    n = 2 * g + j
                    nc.scalar.dma_start(
                        vhi_new[:, 4 * n : 4 * n + 4, :],
                        hiT[64 * g : 64 * (g + 1), :],
                        transpose=True,
                    )
                    nc.scalar.dma_start(
                        vlo_new[:, 4 * n : 4 * n + 4, :],
                        loT[64 * g : 64 * (g + 1), :],
                        transpose=True,
                    )
            nc.vector.tensor_add(y_rm[:], y_rm[:], vhi_new[:])
            nc.vector.tensor_add(y_rm[:], y_rm[:], vlo_new[:])
            vhi, vlo = vhi_new, vlo_new

        nc.sync.dma_start(y_d.rearrange("(k p) c -> p k c", p=128), y_rm[:])

    nc.compile()
    return nc


def _get_nc(t: float):
    key = np.float32(t).tobytes()
    if key not in _cache:
        _cache[key] = _build(t)
    return _cache[key]


def kernel(x: np.ndarray, L: np.ndarray, t: np.ndarray) -> np.ndarray:
    global last_result
    assert x.shape == (N, C) and L.shape == (N, N)
    t_val = float(np.float32(max(float(np.asarray(t).reshape(-1)[0]), 1e-8)))
    nc = _get_nc(t_val)

    L32 = np.ascontiguousarray(L, dtype=np.float32)
    L_hi = L32.astype(BF16)
    L_lo = (L32 - L_hi.astype(np.float32)).astype(BF16)
    x32 = np.ascontiguousarray(x, dtype=np.float32)

    in_maps = [
        {
            "x": np.ascontiguousarray(x32[:, c * CS : (c + 1) * CS]),
            "L_hi": L_hi,
            "L_lo": L_lo,
        }
        for c in range(N_CORES)
    ]
    res = run_bass_kernel_spmd(nc, in_maps, core_ids=list(range(N_CORES)))
    last_result = res
    y = np.concatenate(
        [res.results[c]["y"] for c in range(N_CORES)], axis=1
    ).astype(np.float32)
    return y


# revision 6
# speedup vs baseline: 1.1531x; 1.1531x over previous
"""Trainium2 Bass kernel for nn_Diffusion: y = expm(-t*L) @ x.

Math: ||t*L||_2 ~= 0.2 for the target inputs (L is PSD with eigenvalues
roughly in [0, 0.4], t = 0.5), so the action of the matrix exponential is
computed with a degree-D Taylor series applied as chained matvecs:

    y = sum_{k=0..D} (-t)^k/k! L^k x,   v_0 = x,  v_k = (-t/k) * L @ v_{k-1}

Sharding: x is split column-wise (channel-parallel) across the 8 cores (64
channels each); L is replicated. No cross-core communication.

Per-core compute (transposed orientation, so the PE streams N=512-wide):
    v'^T = v^T @ L  computed as  out = lhsT.T @ rhs  with lhsT = v row-major
    tiles [128, 64] and rhs = L row-blocks [128, 512].
Full fp32 accuracy at bf16 PE speed via a hi/lo split of both operands:
    L = L_hi + L_lo (bf16 pair, host-prepared), v = v_hi + v_lo (bf16 pair),
    L@v ~= L_hi v_hi + L_hi v_lo + L_lo v_hi  (3 bf16 products, fp32 PSUM).
Terms 3+ carry ~1e-3 relative weight, so a single product (L_hi v_hi)
suffices there (verified: total rel err 7e-7, same as all-3-split).
The two free PE column halves are used concurrently (tile_position col
packing): col group g computes output chunk n = 2g + j during "j-pass" j.

Between terms, each j-pass's channel-major result [128, 512] is scaled out
of PSUM (ACT), split to bf16 hi/lo (DVE), and DMA-xbar-transposed back to
row-major [128, 4, 64] tile slices for the next term's stationary operand.
The next term's contraction visits the k-tiles produced by its
predecessor's j0-pass first, so the j1-pass post-processing overlaps the
next term's matmuls. y accumulates row-major in fp32 and is DMA'd out
directly.

x/y cross the HBM boundary in a host-shuffled row order (row p*16+k holds
logical row 128k+p) so every DMA moves 4 KB-contiguous per partition; the
host applies the (free) inverse permutation.
"""

import os
import sys

for _p in ("/opt/trn_rl_repo", "/root/.axon_site/_ro/trn_rl_repo"):
    if os.path.isdir(_p) and _p not in sys.path:
        sys.path.insert(0, _p)

from contextlib import ExitStack

import ml_dtypes
import numpy as np

import concourse.bacc as bacc
import concourse.mybir as mybir
import concourse.tile as tile
from concourse.bass_utils import run_bass_kernel_spmd

BF16 = ml_dtypes.bfloat16
N = 2048
C = 512
N_CORES = 8
CS = C // N_CORES  # 64 channels per core
KT = N // 128  # 16 contraction tiles
D = 4  # Taylor degree (rel err ~7e-7 for t*||L|| ~= 0.2)
CHUNK = 512
FULL_SPLIT_TERMS = 2  # 3-product terms; later terms use L_hi v_hi only

_cache: dict = {}
last_result = None  # BassKernelResults of the most recent run (for test.py)

# k-tile visit order for terms > 1: tiles produced by the previous term's
# j0-pass (rows 0-511 and 1024-1535 -> tiles 0-3 and 8-11) first, so the
# j1-pass transposes can complete while these are being consumed.
KK_BATCH_A = [0, 1, 2, 3, 8, 9, 10, 11]
KK_BATCH_B = [4, 5, 6, 7, 12, 13, 14, 15]


def _build(t: float):
    f32 = mybir.dt.float32
    bf16 = mybir.dt.bfloat16
    nc = bacc.Bacc(
        "TRN2", target_bir_lowering=False, debug=False, num_devices=N_CORES
    )
    x_d = nc.dram_tensor("x", [N, CS], f32, kind="ExternalInput").ap()
    Lhi_d = nc.dram_tensor("L_hi", [N, N], bf16, kind="ExternalInput").ap()
    Llo_d = nc.dram_tensor("L_lo", [N, N], bf16, kind="ExternalInput").ap()
    y_d = nc.dram_tensor("y", [N, CS], f32, kind="ExternalOutput").ap()

    with ExitStack() as ctx:
        tc = ctx.enter_context(tile.TileContext(nc))
        Lp = ctx.enter_context(tc.tile_pool(name="L", bufs=1))
        vp = ctx.enter_context(tc.tile_pool(name="v", bufs=2))
        sp = ctx.enter_context(tc.tile_pool(name="s", bufs=3))
        yp = ctx.enter_context(tc.tile_pool(name="yp", bufs=1))
        pp = ctx.enter_context(tc.tile_pool(name="ps", bufs=2, space="PSUM"))

        Lhi = Lp.tile([128, KT, N], bf16, tag="Lhi")
        Llo = Lp.tile([128, KT, N], bf16, tag="Llo")
        x_sb = yp.tile([128, KT, CS], f32, tag="xsb")
        y_rm = yp.tile([128, KT, CS], f32, tag="y")

        # x arrives host-shuffled: dram row p*16+k = logical row 128k+p, so
        # each partition reads 4 KB contiguous. SWDGE queue keeps it off the
        # L queue.
        nc.gpsimd.dma_start(x_sb[:], x_d.rearrange("(p k) c -> p k c", k=KT))
        # L in 4 MB transfers (8 k-tiles each) to amortize per-DMA latency;
        # L_hi first (most products stream it), then L_lo.
        for Ld, Lsb in ((Lhi_d, Lhi), (Llo_d, Llo)):
            for h in (0, 1):
                nc.sync.dma_start(
                    Lsb[:, 8 * h : 8 * (h + 1), :],
                    Ld[1024 * h : 1024 * (h + 1), :].rearrange(
                        "(k p) c -> p k c", p=128
                    ),
                )

        # v_0 = x as a bf16 hi/lo pair; y starts as the exact fp32 x.
        vhi = vp.tile([128, KT, CS], bf16, tag="vhi")
        vlo = vp.tile([128, KT, CS], bf16, tag="vlo")
        nc.vector.tensor_copy(vhi[:], x_sb[:])
        nc.vector.tensor_sub(vlo[:], x_sb[:], vhi[:])
        nc.scalar.copy(y_rm[:], x_sb[:])

        for k in range(1, D + 1):
            s_k = float(-t / k)
            full = k <= FULL_SPLIT_TERMS
            # products: index into (lhsT, rhs) pairs
            prods = [(vhi, Lhi), (vlo, Lhi), (vhi, Llo)] if full else [(vhi, Lhi)]
            batches = [list(range(KT))] if k == 1 else [KK_BATCH_A, KK_BATCH_B]
            # Emission order. Term 1 defers the L_lo product of both
            # j-passes to the end so the PE is not blocked on the L_lo DMA.
            if k == 1 and full:
                order = [
                    (j, pi, kk)
                    for pis in ([0, 1], [2])
                    for j in (0, 1)
                    for pi in pis
                    for batch in batches
                    for kk in batch
                ]
            else:
                order = [
                    (j, pi, kk)
                    for j in (0, 1)
                    for batch in batches
                    for pi in range(len(prods))
                    for kk in batch
                ]
            # first/last MM per j (flags apply to both col-group regions,
            # which see the same (j, pi, kk) sequence).
            per_j = {j: [i for i, o in enumerate(order) if o[0] == j] for j in (0, 1)}
            first = {j: min(ix) for j, ix in per_j.items()}
            last = {j: max(ix) for j, ix in per_j.items()}

            ps = {
                j: pp.tile(
                    [128, CHUNK], f32, tag=f"ps{j}", name=f"ps{j}_{k}"
                )
                for j in (0, 1)
            }
            vhi_new = vp.tile([128, KT, CS], bf16, tag="vhi")
            vlo_new = vp.tile([128, KT, CS], bf16, tag="vlo")

            done_j = set()
            for idx, (j, pi, kk) in enumerate(order):
                vt, Lt = prods[pi]
                for g in (0, 1):
                    n = 2 * g + j
                    nc.tensor.matmul(
                        ps[j][64 * g : 64 * (g + 1), :],
                        vt[:, kk, :],
                        Lt[:, kk, CHUNK * n : CHUNK * (n + 1)],
                        start=(idx == first[j]),
                        stop=(idx == last[j]),
                        tile_position=(0, 64 * g),
                        # The two col-groups' accumulation groups share a
                        # PSUM bank on disjoint partition ranges; the sim's
                        # zero-region tracker is partition-blind.
                        skip_group_check=True,
                    )
                if idx == last[j] and j not in done_j:
                    done_j.add(j)
                    # Post-process pass j: scale out of PSUM, split to bf16
                    # hi/lo (channel-major), transpose to row-major.
                    yT = sp.tile([128, CHUNK], f32, tag="yT")
                    nc.scalar.mul(yT[:], ps[j][:], s_k)
                    hiT = sp.tile([128, CHUNK], bf16, tag="hiT")
                    loT = sp.tile([128, CHUNK], bf16, tag="loT")
                    nc.vector.tensor_copy(hiT[:], yT[:])
                    nc.vector.tensor_sub(loT[:], yT[:], hiT[:])
                    for g in (0, 1):
                        n = 2 * g + j
                        nc.scalar.dma_start(
                            vhi_new[:, 4 * n : 4 * n + 4, :],
                            hiT[64 * g : 64 * (g + 1), :],
                            transpose=True,
                        )
                        nc.scalar.dma_start(
                            vlo_new[:, 4 * n : 4 * n + 4, :],
                            loT[64 * g : 64 * (g + 1), :],
                            transpose=True,
                        )
            nc.vector.tensor_add(y_rm[:], y_rm[:], vhi_new[:])
            nc.vector.tensor_add(y_rm[:], y_rm[:], vlo_new[:])
            vhi, vlo = vhi_new, vlo_new

        # y leaves host-shuffled (row p*16+k = logical row 128k+p): 4 KB
        # contiguous per partition; host inverts the permutation.
        nc.sync.dma_start(y_d.rearrange("(p k) c -> p k c", k=KT), y_rm[:])

    nc.compile()
    return nc


def _get_nc(t: float):
    key = np.float32(t).tobytes()
    if key not in _cache:
        _cache[key] = _build(t)
    return _cache[key]


def kernel(x: np.ndarray, L: np.ndarray, t: np.ndarray) -> np.ndarray:
    global last_result
    assert x.shape == (N, C) and L.shape == (N, N)
    t_val = float(np.float32(max(float(np.asarray(t).reshape(-1)[0]), 1e-8)))
    nc = _get_nc(t_val)

    L32 = np.ascontiguousarray(L, dtype=np.float32)
    L_hi = L32.astype(BF16)
    L_lo = (L32 - L_hi.astype(np.float32)).astype(BF16)
    x32 = np.ascontiguousarray(x, dtype=np.float32)

    in_maps = []
    for c in range(N_CORES):
        slab = x32[:, c * CS : (c + 1) * CS]
        # device row order: row p*16+k holds logical row 128k+p
        x_shuf = np.ascontiguousarray(
            slab.reshape(KT, 128, CS).transpose(1, 0, 2).reshape(N, CS)
        )
        in_maps.append({"x": x_shuf, "L_hi": L_hi, "L_lo": L_lo})
    res = run_bass_kernel_spmd(nc, in_maps, core_ids=list(range(N_CORES)))
    last_result = res
    outs = []
    for c in range(N_CORES):
        y_dev = res.results[c]["y"]
        outs.append(
            y_dev.reshape(128, KT, CS).transpose(1, 0, 2).reshape(N, CS)
        )
    return np.concatenate(outs, axis=1).astype(np.float32)
